# revision 34
# baseline (speedup 1.0000x reference)
import os
import sys

sys.path.insert(0, "/opt/trn_rl_repo")

import numpy as np
import ml_dtypes

import concourse.bass as bass
import concourse.bacc as bacc
import concourse.mybir as mybir
from concourse.bass_utils import run_bass_kernel_spmd
from concourse.tile import TileContext

S = 1024
DIM = 2560
HD = 128
NH = 20
NKV = 5
GS = 128
THETA = 500000.0
EPS = 1e-05
KBASE = NH * HD            # k rows start in w_qkv
VBASE = KBASE + NKV * HD   # v rows start
NC = 8
KCH = DIM // 128           # 20 k-chunks
WQCOLS = 7 * 128           # [qs0 qs1 qs2 kA vA kB vB]
OC = DIM // NC             # 320 output cols per core
MT = S // 128              # 8 token tiles

# head assignment per core: [slot0, slot1, slot2]; None = garbage slot
HEADS = [
    [0, 1, 8], [2, 3, 9], [4, 5, 10], [6, 7, 11],
    [12, 13, None], [14, 15, None], [16, 17, None], [18, 19, None],
]
GA = [0, 0, 1, 1, 3, 3, 4, 4]              # kv group for slots 0,1
GB = [2, 2, 2, 2, None, None, None, None]  # kv group for slot 2
REAL_CHUNKS = [j * 3 + s for j in range(NC) for s in range(3) if HEADS[j][s] is not None]
assert len(REAL_CHUNKS) == NH

FP16 = np.float16
SCALE = float(HD) ** -0.5
ESHIFT = -2.0  # exp(score*SCALE + ESHIFT); cancels in softmax ratio

_cached = {}


def _build_nc():
    nc = bacc.Bacc("TRN2", target_bir_lowering=False, debug=False, num_devices=NC)
    f32 = mybir.dt.float32
    f16 = mybir.dt.float16

    # host-prequantized activations, already transposed: [dim, tok] fp16
    q8t_d = nc.declare_dram_parameter("q8t", [DIM, S], f16, isOutput=False)
    # per-token 1/s dequant scales: [tok%128, tok//128]
    rs_d = nc.declare_dram_parameter("rs", [128, MT], f32, isOutput=False)
    wq_d = nc.declare_dram_parameter("wq", [DIM, WQCOLS], f16, isOutput=False)
    wo_d = nc.declare_dram_parameter("wo", [NC * 384, OC], f16, isOutput=False)
    tq1_d = nc.declare_dram_parameter("tq1", [S, HD], f16, isOutput=False)
    tq2_d = nc.declare_dram_parameter("tq2", [S, HD], f16, isOutput=False)
    tk1_d = nc.declare_dram_parameter("tk1", [S, HD], f16, isOutput=False)
    tk2_d = nc.declare_dram_parameter("tk2", [S, HD], f16, isOutput=False)
    ident_d = nc.declare_dram_parameter("ident", [128, 128], f16, isOutput=False)
    # 4 causal mask variants for 512-wide score groups: r = kc - 4*grp
    cmask_d = nc.declare_dram_parameter("cmask", [4 * 128, 512], f16, isOutput=False)
    out_d = nc.declare_dram_parameter("out", [S, OC], f32, isOutput=True)

    warm_in = nc.dram_tensor("warmin", [16, 16], f16, kind="Internal")
    warm_out = nc.dram_tensor("warmout", [NC * 16, 16], f16, kind="Internal",
                              addr_space="Shared")
    # attention outputs gathered per token half (a = grp0, b = grp1)
    agin_a = nc.dram_tensor("agina", [384, S // 2], f16, kind="Internal")
    agin_b = nc.dram_tensor("aginb", [384, S // 2], f16, kind="Internal")
    agout_a = nc.dram_tensor("agouta", [NC * 384, S // 2], f16, kind="Internal",
                             addr_space="Shared")
    agout_b = nc.dram_tensor("agoutb", [NC * 384, S // 2], f16, kind="Internal",
                             addr_space="Shared")

    with TileContext(nc) as tc:
        with (
            tc.tile_pool(name="cst", bufs=1) as cst,
            tc.tile_pool(name="kvsb", bufs=1) as kvsb,
            tc.tile_pool(name="nrp", bufs=2) as nrp,
        ):
            ones_col = cst.tile([128, 1], f16, tag="onesc", name="onesc")
            nc.vector.memset(ones_col[:, :], 1.0)
            eshift = cst.tile([128, 1], f32, tag="esh", name="esh")
            nc.vector.memset(eshift[:, :], ESHIFT)
            epsT = cst.tile([128, 1], f32, tag="eps", name="eps")
            nc.vector.memset(epsT[:, :], EPS)

            # Warmup collective: pays the ncfw cold-start + launch-skew
            # barrier while qkv runs, so the attention-output AllGathers
            # enter the mesh hot. No data deps; transfers garbage.
            nc.gpsimd.collective_compute(
                "AllGather", mybir.AluOpType.bypass,
                ins=[warm_in.ap().opt()], outs=[warm_out.ap().opt()],
                replica_groups=[list(range(NC))],
            )

            rs_cols = cst.tile([128, MT], f32, tag="rscols", name="rscols")
            nc.gpsimd.dma_start(out=rs_cols[:, :], in_=rs_d[:, :])

            # ---- bulk loads, grouped 4 chunks per DMA, interleaved across
            # the three DMA-capable queues so chunk kc=0 lands first.
            # q8/wq live in their own pool that closes after stage C so the
            # o_proj input tiles can reuse that SBUF.
            ldp = tc.tile_pool(name="ldp", bufs=1)
            ldp_pool = ldp.__enter__()
            NG = KCH // 4
            q8g = [ldp_pool.tile([128, 4, S], f16, tag=f"q8g{g}", name=f"q8g{g}")
                   for g in range(NG)]
            wqg = [ldp_pool.tile([128, 4, WQCOLS], f16, tag=f"wqg{g}", name=f"wqg{g}")
                   for g in range(NG)]

            def q8ap(g):
                return q8t_d.ap()[g * 512:(g + 1) * 512, :].rearrange(
                    "(c p) s -> p c s", p=128)

            def wqap(g):
                return wq_d.ap()[g * 512:(g + 1) * 512, :].rearrange(
                    "(c p) s -> p c s", p=128)

            # sync: q8 kc0 first (single chunk), then the rest of group 0
            nc.sync.dma_start(out=q8g[0][:, 0, :], in_=q8ap(0)[:, 0, :])
            nc.scalar.dma_start(out=wqg[0][:, 0, :], in_=wqap(0)[:, 0, :])
            nc.sync.dma_start(out=q8g[0][:, 1:4, :], in_=q8ap(0)[:, 1:4, :])
            nc.scalar.dma_start(out=wqg[0][:, 1:4, :], in_=wqap(0)[:, 1:4, :])
            nc.sync.dma_start(out=q8g[1][:, :, :], in_=q8ap(1))
            nc.gpsimd.dma_start(out=q8g[2][:, :, :], in_=q8ap(2))
            nc.scalar.dma_start(out=wqg[1][:, :, :], in_=wqap(1))
            nc.sync.dma_start(out=q8g[3][:, :, :], in_=q8ap(3))
            nc.gpsimd.dma_start(out=q8g[4][:, :, :], in_=q8ap(4))
            nc.scalar.dma_start(out=wqg[2][:, :, :], in_=wqap(2))
            nc.scalar.dma_start(out=wqg[3][:, :, :], in_=wqap(3))
            nc.scalar.dma_start(out=wqg[4][:, :, :], in_=wqap(4))

            def q8c(kc):
                return q8g[kc // 4][:, kc % 4, :]

            def wqc(kc):
                return wqg[kc // 4][:, kc % 4, :]

            # rope tables / identity / mask on sync, behind the q8 stream;
            # tables are token-major (rope runs pre-transpose in [tok, d])
            tabs = {}
            for nm, d in (("tq1", tq1_d), ("tk1", tk1_d),
                          ("tq2", tq2_d), ("tk2", tk2_d)):
                t = cst.tile([128, MT, HD], f16, tag=f"tb{nm}", name=f"tb{nm}")
                nc.sync.dma_start(out=t[:, :, :],
                                  in_=d.ap().rearrange("(m p) d -> p m d", p=128))
                tabs[nm] = t
            ident = cst.tile([128, 128], f16, tag="id", name="id")
            nc.sync.dma_start(out=ident[:, :], in_=ident_d[:, :])
            cmask = cst.tile([128, 4, 512], f16, tag="cm", name="cm")
            nc.sync.dma_start(out=cmask[:, :, :],
                              in_=cmask_d.ap().rearrange("(r p) n -> p r n", p=128))
            # o_proj weights: not needed until late; tail of scalar queue
            wog = [cst.tile([128, 12, OC], f16, tag=f"wog{g}", name=f"wog{g}")
                   for g in range(2)]
            for g in range(2):
                nc.scalar.dma_start(
                    out=wog[g][:, :, :],
                    in_=wo_d.ap()[g * 1536:(g + 1) * 1536, :].rearrange(
                        "(c p) s -> p c s", p=128))

            def woc(ck):
                return wog[ck // 12][:, ck % 12, :]

            # persistent roped q/k: [d, slot(q0 q1 q2 kA kB), tok]
            qkT = kvsb.tile([128, 5, S], f16, tag="qkT", name="qkT")
            VV = [kvsb.tile([128, 2, 128], f16, tag=f"V{m}", name=f"V{m}")
                  for m in range(MT)]
            # per-k-token exp scale SCALE*rsqrt(ms_k): k rms norm commutes
            # with rope, so it rides the attention exp's per-partition scale
            kscal = [kvsb.tile([128, 2], f32, tag=f"ks{m}", name=f"ks{m}")
                     for m in range(MT)]

            def norm_rope_batched(eng, xn_view, t1, t2, ob_view, scratch_tag):
                """xn_view [128, nh, 128] f16 normalized input in d-permuted
                layout (even dims in cols 0:64, odd in 64:128); t1/t2 f16
                [128, 128] split-table column slices for this m-tile;
                writes roped f16 [128, nh, 128] in the same layout."""
                nh = xn_view.shape[1]
                x0 = xn_view[:, :, 0:64]
                x1 = xn_view[:, :, 64:128]
                t1b = t1.rearrange("p (one d) -> p one d", one=1).to_broadcast(
                    [128, nh, HD])
                t2b = t2.rearrange("p (one d) -> p one d", one=1).to_broadcast(
                    [128, nh, HD])
                a1 = nrp.tile([128, nh, 64], f16, tag=f"ra1{scratch_tag}",
                              name=f"ra1{scratch_tag}")
                a2 = nrp.tile([128, nh, 64], f16, tag=f"ra2{scratch_tag}",
                              name=f"ra2{scratch_tag}")
                eng.tensor_mul(a1[:, :, :], x0, t1b[:, :, 0:64])
                eng.tensor_mul(a2[:, :, :], x1, t2b[:, :, 64:128])
                eng.tensor_sub(ob_view[:, :, 0:64], a1[:, :, :], a2[:, :, :])
                eng.tensor_mul(a1[:, :, :], x0, t2b[:, :, 0:64])
                eng.tensor_mul(a2[:, :, :], x1, t1b[:, :, 64:128])
                eng.tensor_add(ob_view[:, :, 64:128], a1[:, :, :], a2[:, :, :])

            def tcol(nm, m):
                return tabs[nm][:, m, :]

            with (
                tc.tile_pool(name="psq", bufs=3, space="PSUM") as psq,
                tc.tile_pool(name="pst", bufs=2, space="PSUM") as pstp,
            ):
                # ---- Stage C: qkv matmul + norm/rope epilogue + PE
                # transpose into qkT. ACT does ONLY Rsqrt here (evacs are
                # on DVE/gpsimd) so no table thrash.
                def qkv_epilogue(m, psA, psB):
                    rs_ap = rs_cols[:, m:m + 1]
                    psBr = psB.rearrange("p (b c) -> p b c", c=256)
                    qxs = nrp.tile([128, 384], f32, tag="qxs", name="qxs")
                    nc.vector.tensor_copy(qxs[:, :], psA[:, :])
                    kxs = nrp.tile([128, 2, 128], f32, tag="kxs", name="kxs")
                    nc.vector.tensor_copy(kxs[:, :, :], psBr[:, :, 0:128])
                    sq = nrp.tile([128, 384], f32, tag="sqq", name="sqq")
                    sk = nrp.tile([128, 256], f32, tag="sqk", name="sqk")
                    nc.vector.tensor_mul(sq[:, :], qxs[:, :], qxs[:, :])
                    nc.vector.tensor_mul(sk[:, :], kxs.rearrange("p b c -> p (b c)"),
                                         kxs.rearrange("p b c -> p (b c)"))
                    rs5 = nrp.tile([128, 5], f32, tag="rs5", name="rs5")
                    nc.vector.tensor_reduce(rs5[:, 0:3],
                                            sq.rearrange("p (h d) -> p h d", d=128),
                                            mybir.AxisListType.X, mybir.AluOpType.add)
                    nc.vector.tensor_reduce(rs5[:, 3:5],
                                            sk.rearrange("p (h d) -> p h d", d=128),
                                            mybir.AxisListType.X, mybir.AluOpType.add)
                    # rsqrt(ms/HD + eps): DVE fast reciprocal + ACT Sqrt
                    nc.vector.tensor_scalar(rs5[:, :], rs5[:, :], 1.0 / HD, EPS,
                                            mybir.AluOpType.mult,
                                            mybir.AluOpType.add)
                    rc5 = nrp.tile([128, 5], f32, tag="rc5", name="rc5")
                    nc.vector.reciprocal_approx_fast(rc5[:, :], rs5[:, :])
                    nc.scalar.activation(rs5[:, :], rc5[:, :],
                                         mybir.ActivationFunctionType.Sqrt)
                    nc.vector.tensor_scalar_mul(kscal[m][:, :], rs5[:, 3:5], SCALE)
                    # q norm, V scale, and k cast all ride ACT Copy-with-scale
                    # (the DVE is contended during the qkv matmul stream)
                    nc.scalar.activation(VV[m][:, :, :], psBr[:, :, 128:256],
                                         mybir.ActivationFunctionType.Copy,
                                         scale=rs_ap)
                    qx16 = nrp.tile([128, 3, 128], f16, tag="qx16", name="qx16")
                    for h in range(3):
                        nc.scalar.activation(qx16[:, h, :],
                                             qxs[:, h * 128:(h + 1) * 128],
                                             mybir.ActivationFunctionType.Copy,
                                             scale=rs5[:, h:h + 1])
                    kx16 = nrp.tile([128, 2, 128], f16, tag="kx16", name="kx16")
                    nc.scalar.copy(kx16[:, :, :], kxs[:, :, :])
                    rbq = nrp.tile([128, 5, HD], f16, tag="rbq", name="rbq")
                    norm_rope_batched(nc.vector, qx16[:, :, :],
                                      tcol("tq1", m), tcol("tq2", m),
                                      rbq[:, 0:3, :], "q")
                    norm_rope_batched(nc.gpsimd, kx16[:, :, :],
                                      tcol("tk1", m), tcol("tk2", m),
                                      rbq[:, 3:5, :], "k")
                    return rbq

                def transpose_m(m, rbq):
                    # PE transpose [tok, d] -> [d, tok] for the 5 slots,
                    # then one strided DVE evac into qkT columns
                    pst = pstp.tile([128, 5, 128], f16, tag="pst", name="pst")
                    for sl in range(5):
                        nc.tensor.transpose(pst[:, sl, :], rbq[:, sl, :],
                                            ident[:, :])
                    nc.vector.tensor_copy(qkT[:, :, m * 128:(m + 1) * 128],
                                          pst[:, :, :])

                rbqs = {}
                # kc-outer over m0-2 (consumes q8/wq chunks as they arrive)
                psA3 = [psq.tile([128, 384], f32, tag="psA", name="psA")
                        for _ in range(3)]
                psB3 = [psq.tile([128, 512], f32, tag="psB", name="psB")
                        for _ in range(3)]
                for kc in range(KCH):
                    for m in range(3):
                        lh = q8c(kc)[:, m * 128:(m + 1) * 128]
                        nc.tensor.matmul(psA3[m][:, :], lh, wqc(kc)[:, 0:384],
                                         start=(kc == 0), stop=(kc == KCH - 1))
                        nc.tensor.matmul(psB3[m][:, :], lh, wqc(kc)[:, 384:896],
                                         start=(kc == 0), stop=(kc == KCH - 1))
                for m in range(3):
                    rbqs[m] = qkv_epilogue(m, psA3[m], psB3[m])
                # kc-inner for m3-7, transposes of earlier tiles interleaved
                # so the PE never waits on a rope chain
                TSCHED = {3: [0], 4: [1, 2], 5: [3], 6: [4], 7: [5]}
                for m in range(3, MT):
                    psA = psq.tile([128, 384], f32, tag="psA", name="psA")
                    psB = psq.tile([128, 512], f32, tag="psB", name="psB")
                    for kc in range(KCH):
                        lh = q8c(kc)[:, m * 128:(m + 1) * 128]
                        nc.tensor.matmul(psA[:, :], lh, wqc(kc)[:, 0:384],
                                         start=(kc == 0), stop=(kc == KCH - 1))
                        nc.tensor.matmul(psB[:, :], lh, wqc(kc)[:, 384:896],
                                         start=(kc == 0), stop=(kc == KCH - 1))
                    for tm in TSCHED[m]:
                        transpose_m(tm, rbqs.pop(tm))
                    rbqs[m] = qkv_epilogue(m, psA, psB)
                for m in sorted(rbqs):
                    transpose_m(m, rbqs.pop(m))
            ldp.__exit__(None, None, None)

            # ---- Stage F: attention, 512-wide q groups, scoresT [k, q];
            # grp0 (tokens 0-511) first so its AllGather fires early and
            # o_proj-A overlaps AG_b's mesh. ACT does ONLY Exp here; 1/den
            # via DVE reciprocal_approx_fast; broadcast via gpsimd.
            with (
                tc.tile_pool(name="pssc", bufs=4, space="PSUM") as pssc,
                tc.tile_pool(name="psav", bufs=2, space="PSUM") as psav,
                tc.tile_pool(name="psden", bufs=1, space="PSUM") as psden,
                tc.tile_pool(name="pso", bufs=1, space="PSUM") as pso,
                tc.tile_pool(name="ptt", bufs=12) as ptt,
                tc.tile_pool(name="accp", bufs=2) as accp,
                tc.tile_pool(name="qga", bufs=2) as qga,
                tc.tile_pool(name="agp", bufs=1) as agp,
                tc.tile_pool(name="ogp", bufs=2) as ogp,
            ):
                agt_a = [agp.tile([128, 12, 512], f16, tag=f"aga{g}",
                                  name=f"aga{g}") for g in range(2)]
                agt_b = [agp.tile([128, 12, 512], f16, tag=f"agb{g}",
                                  name=f"agb{g}") for g in range(2)]
                for grp in (0, 1):
                    gs = slice(grp * 512, grp * 512 + 512)
                    nkc = 4 * grp + 4
                    for sl in range(3):
                        blk = 0 if sl < 2 else 1
                        pts = []
                        den_ps = psden.tile([1, 512], f32, tag="den", name="den")
                        # all score matmuls first — exps chase them via psum
                        # recycling, so the PE never waits on the exp chain
                        for kc in range(nkc):
                            ps = pssc.tile([128, 512], f32, tag="sc", name="sc")
                            nc.tensor.matmul(ps[:, :],
                                             qkT[:, 3 + blk, kc * 128:(kc + 1) * 128],
                                             qkT[:, sl, gs], start=True, stop=True)
                            pt = ptt.tile([128, 512], f16, tag="pt", name="pt")
                            nc.scalar.activation(pt[:, :], ps[:, :],
                                                 mybir.ActivationFunctionType.Exp,
                                                 bias=eshift[:, 0:1],
                                                 scale=kscal[kc][:, blk:blk + 1])
                            r = kc - 4 * grp
                            if r >= 0:
                                nc.vector.tensor_mul(pt[:, :], pt[:, :], cmask[:, r, :])
                            pts.append(pt)
                        avp = psav.tile([128, 512], f32, tag="av", name="av")
                        for kc in range(nkc):
                            nc.tensor.matmul(den_ps[:, :], ones_col[:, :],
                                             pts[kc][:, :],
                                             start=(kc == 0), stop=(kc == nkc - 1))
                            nc.tensor.matmul(avp[:, :], VV[kc][:, blk, :],
                                             pts[kc][:, :],
                                             start=(kc == 0), stop=(kc == nkc - 1))
                        # clamp away 0/denorm (undefined for the fast recip)
                        dsb = accp.tile([1, 512], f32, tag="dsb", name="dsb")
                        nc.vector.tensor_scalar_max(dsb[:, :], den_ps[:, :], 1e-20)
                        rden = accp.tile([1, 512], f32, tag="rdr", name="rdr")
                        nc.vector.reciprocal_approx_fast(rden[:, :], dsb[:, :])
                        fac = qga.tile([128, 512], f32, tag="fac", name="fac")
                        nc.gpsimd.partition_broadcast(fac[:, :], rden[:, :])
                        aq = qga.tile([128, 512], f16, tag="aq", name="aq")
                        nc.vector.tensor_mul(aq[:, :], avp[:, :], fac[:, :])
                        # aq outputs ride the gpsimd queue (same as the AG
                        # triggers) so agt loads on sync/scalar can't delay
                        # this core's collective inputs
                        agin = agin_b if grp == 1 else agin_a
                        nc.gpsimd.dma_start(
                            out=agin[sl * 128:(sl + 1) * 128, :],
                            in_=aq[:, :])
                    nc.gpsimd.collective_compute(
                        "AllGather", mybir.AluOpType.bypass,
                        ins=[(agin_b if grp == 1 else agin_a).ap().opt()],
                        outs=[(agout_b if grp == 1 else agout_a).ap().opt()],
                        replica_groups=[list(range(NC))],
                    )
                    # issue half-A o_proj input loads right behind its AG
                    # trigger so they land the moment the mesh finishes.
                    # Half-B loads are deferred past the half-A matmuls:
                    # queue-position DMA semaphores would otherwise make
                    # o_proj-A wait for AG_b's mesh.
                    if grp == 0:
                        for g in range(2):
                            eng = nc.sync if g == 0 else nc.scalar
                            eng.dma_start(
                                out=agt_a[g][:, :, :],
                                in_=agout_a.ap()[g * 1536:(g + 1) * 1536, :]
                                .rearrange("(c p) s -> p c s", p=128))

                # ---- o_proj: half A (tokens 0-511) first — its inputs
                # arrived during grp1 attention; half B rides AG_b.
                def oproj_half(hf, agt):
                    for j in range(4):
                        m = hf * 4 + j
                        ps = pso.tile([128, OC], f32, tag="po", name="po")
                        for i, ck in enumerate(REAL_CHUNKS):
                            nc.tensor.matmul(ps[:, :],
                                             agt[ck // 12][:, ck % 12,
                                                           j * 128:(j + 1) * 128],
                                             woc(ck)[:, :],
                                             start=(i == 0),
                                             stop=(i == NH - 1))
                        og = ogp.tile([128, OC], f32, tag="og", name="og")
                        nc.scalar.copy(og[:, :], ps[:, :])
                        nc.sync.dma_start(out=out_d[m * 128:(m + 1) * 128, :],
                                          in_=og[:, :])

                oproj_half(0, agt_a)
                for g in range(2):
                    eng = nc.sync if g == 0 else nc.scalar
                    eng.dma_start(
                        out=agt_b[g][:, :, :],
                        in_=agout_b.ap()[g * 1536:(g + 1) * 1536, :]
                        .rearrange("(c p) s -> p c s", p=128))
                oproj_half(1, agt_b)

    nc.compile()
    return nc


def _host_prep(x, w_qkv, ws_qkv, w_o, ws_o, q_norm_w, k_norm_w):
    w_dq = (w_qkv * np.repeat(ws_qkv, GS, axis=1)).astype(np.float32)
    wo_dq = (w_o * np.repeat(ws_o, GS, axis=1)).astype(np.float32)

    # per-token int8 activation quant (reference's _quant_act), transposed
    # to [dim, tok] so the device consumes it without DMA transposes
    mx = np.clip(np.max(np.abs(x), axis=-1, keepdims=True), 1e-5, None)
    q8 = np.clip(np.round(x * (127.0 / mx)), -128.0, 127.0)
    q8t = np.ascontiguousarray(q8.T).astype(FP16)            # [DIM, S]
    rs = np.ascontiguousarray(
        (mx[:, 0] / 127.0).reshape(MT, 128).T).astype(np.float32)  # [128, MT]

    pos = np.arange(S, dtype=np.float32)
    inv_freq = (THETA ** (-np.arange(0, HD, 2, dtype=np.float32) / HD)).astype(np.float32)
    ang = pos[:, None] * inv_freq[None, :]
    co = np.cos(ang).astype(np.float32)                  # [S, 64]
    si = np.sin(ang).astype(np.float32)
    # split rope tables for the d-permuted (even|odd) head layout:
    # t1 = [c*w_even | c*w_odd], t2 = [s*w_even | s*w_odd]
    tq1 = np.concatenate([co * q_norm_w[0::2], co * q_norm_w[1::2]], 1).astype(FP16)
    tq2 = np.concatenate([si * q_norm_w[0::2], si * q_norm_w[1::2]], 1).astype(FP16)
    tk1 = np.concatenate([co * k_norm_w[0::2], co * k_norm_w[1::2]], 1).astype(FP16)
    tk2 = np.concatenate([si * k_norm_w[0::2], si * k_norm_w[1::2]], 1).astype(FP16)
    # even dims then odd dims within each q/k head (scores are invariant
    # since q and k share the permutation; v / o_proj stay unpermuted)
    dperm = np.concatenate([np.arange(0, HD, 2), np.arange(1, HD, 2)])

    ident = np.eye(128, dtype=FP16)

    # mask variants: scoresT [k(128), 512 q]; group cols = 4 q-blocks; r = kc-4*grp
    cm = np.zeros((4, 128, 512), np.float32)
    tri = np.triu(np.ones((128, 128), np.float32))  # keep k <= q
    for r in range(4):
        for j in range(4):
            if j > r:
                cm[r, :, j * 128:(j + 1) * 128] = 1.0
            elif j == r:
                cm[r, :, j * 128:(j + 1) * 128] = tri
    cmask = cm.reshape(4 * 128, 512).astype(FP16)

    in_maps = []
    for c in range(NC):
        wq = np.zeros((DIM, WQCOLS), np.float32)
        for sl in range(3):
            h = HEADS[c][sl]
            if h is not None:
                wq[:, sl * 128:(sl + 1) * 128] = w_dq[h * HD + dperm, :].T
        ga = GA[c]
        wq[:, 384:512] = w_dq[KBASE + ga * HD + dperm, :].T
        wq[:, 512:640] = w_dq[VBASE + ga * HD:VBASE + (ga + 1) * HD, :].T
        gb = GB[c]
        if gb is not None:
            wq[:, 640:768] = w_dq[KBASE + gb * HD + dperm, :].T
            wq[:, 768:896] = w_dq[VBASE + gb * HD:VBASE + (gb + 1) * HD, :].T

        wo = np.zeros((NC * 384, OC), np.float32)
        for j in range(NC):
            for sl in range(3):
                h = HEADS[j][sl]
                if h is not None:
                    rws = slice((j * 3 + sl) * 128, (j * 3 + sl) * 128 + 128)
                    wo[rws, :] = wo_dq[c * OC:(c + 1) * OC, h * HD:(h + 1) * HD].T

        in_maps.append({
            "q8t": q8t,
            "rs": rs,
            "wq": wq.astype(FP16),
            "wo": wo.astype(FP16),
            "tq1": tq1, "tq2": tq2, "tk1": tk1, "tk2": tk2,
            "ident": ident,
            "cmask": cmask,
        })
    return in_maps


def kernel(x, w_qkv, ws_qkv, w_o, ws_o, q_norm_w, k_norm_w):
    x = np.asarray(x, np.float32)
    w_qkv = np.asarray(w_qkv, np.float32)
    ws_qkv = np.asarray(ws_qkv, np.float32)
    w_o = np.asarray(w_o, np.float32)
    ws_o = np.asarray(ws_o, np.float32)
    q_norm_w = np.asarray(q_norm_w, np.float32)
    k_norm_w = np.asarray(k_norm_w, np.float32)

    if "nc" not in _cached:
        _cached["nc"] = _build_nc()
    nc = _cached["nc"]

    in_maps = _host_prep(x, w_qkv, ws_qkv, w_o, ws_o, q_norm_w, k_norm_w)
    trace = bool(int(os.environ.get("BENCH_TRACE", "0")))
    res = run_bass_kernel_spmd(nc, in_maps, core_ids=list(range(NC)), trace=trace)
    _cached["res"] = res
    if trace and res.exec_time_ns is not None:
        print(f"HW exec time: {res.exec_time_ns} ns")
        _cached["exec_time_ns"] = res.exec_time_ns

    out = np.concatenate([np.asarray(res.results[c]["out"], np.float32)
                          for c in range(NC)], axis=1)
    return out


# revision 37
# speedup vs baseline: 1.0320x; 1.0320x over previous
import os
import sys

sys.path.insert(0, "/opt/trn_rl_repo")

import numpy as np
import ml_dtypes

import concourse.bass as bass
import concourse.bacc as bacc
import concourse.mybir as mybir
from concourse.bass_utils import run_bass_kernel_spmd
from concourse.tile import TileContext

S = 1024
DIM = 2560
HD = 128
NH = 20
NKV = 5
GS = 128
THETA = 500000.0
EPS = 1e-05
KBASE = NH * HD            # k rows start in w_qkv
VBASE = KBASE + NKV * HD   # v rows start
NC = 8
KCH = DIM // 128           # 20 k-chunks
WQCOLS = 7 * 128           # [qs0 qs1 qs2 kA vA kB vB]
OC = DIM // NC             # 320 output cols per core
MT = S // 128              # 8 token tiles

# head assignment per core: [slot0, slot1, slot2]; None = garbage slot
HEADS = [
    [0, 1, 8], [2, 3, 9], [4, 5, 10], [6, 7, 11],
    [12, 13, None], [14, 15, None], [16, 17, None], [18, 19, None],
]
GA = [0, 0, 1, 1, 3, 3, 4, 4]              # kv group for slots 0,1
GB = [2, 2, 2, 2, None, None, None, None]  # kv group for slot 2
REAL_CHUNKS = [j * 3 + s for j in range(NC) for s in range(3) if HEADS[j][s] is not None]
assert len(REAL_CHUNKS) == NH

FP16 = np.float16
SCALE = float(HD) ** -0.5
ESHIFT = -2.0  # exp(score*SCALE + ESHIFT); cancels in softmax ratio

_cached = {}


def _build_nc():
    nc = bacc.Bacc("TRN2", target_bir_lowering=False, debug=False, num_devices=NC)
    f32 = mybir.dt.float32
    f16 = mybir.dt.float16

    # host-prequantized activations, already transposed: [dim, tok] fp16
    q8t_d = nc.declare_dram_parameter("q8t", [DIM, S], f16, isOutput=False)
    # per-token 1/s dequant scales: [tok%128, tok//128]
    rs_d = nc.declare_dram_parameter("rs", [128, MT], f32, isOutput=False)
    wq_d = nc.declare_dram_parameter("wq", [DIM, WQCOLS], f16, isOutput=False)
    wo_d = nc.declare_dram_parameter("wo", [NC * 384, OC], f16, isOutput=False)
    tq1_d = nc.declare_dram_parameter("tq1", [S, HD], f16, isOutput=False)
    tq2_d = nc.declare_dram_parameter("tq2", [S, HD], f16, isOutput=False)
    tk1_d = nc.declare_dram_parameter("tk1", [S, HD], f16, isOutput=False)
    tk2_d = nc.declare_dram_parameter("tk2", [S, HD], f16, isOutput=False)
    ident_d = nc.declare_dram_parameter("ident", [128, 128], f16, isOutput=False)
    # 4 causal mask variants for 512-wide score groups: r = kc - 4*grp
    cmask_d = nc.declare_dram_parameter("cmask", [4 * 128, 512], f16, isOutput=False)
    out_d = nc.declare_dram_parameter("out", [S, OC], f32, isOutput=True)

    warm_in = nc.dram_tensor("warmin", [16, 16], f16, kind="Internal")
    warm_out = nc.dram_tensor("warmout", [NC * 16, 16], f16, kind="Internal",
                              addr_space="Shared")
    # attention outputs gathered per token half: a1 = grp0 slots 0-1,
    # a2 = grp0 slot 2, b = grp1 all slots
    agin_a1 = nc.dram_tensor("agina1", [256, S // 2], f16, kind="Internal")
    agin_a2 = nc.dram_tensor("agina2", [128, S // 2], f16, kind="Internal")
    agin_b = nc.dram_tensor("aginb", [384, S // 2], f16, kind="Internal")
    agout_a1 = nc.dram_tensor("agouta1", [NC * 256, S // 2], f16, kind="Internal",
                              addr_space="Shared")
    agout_a2 = nc.dram_tensor("agouta2", [NC * 128, S // 2], f16, kind="Internal",
                              addr_space="Shared")
    agout_b = nc.dram_tensor("agoutb", [NC * 384, S // 2], f16, kind="Internal",
                             addr_space="Shared")

    with TileContext(nc) as tc:
        with (
            tc.tile_pool(name="cst", bufs=1) as cst,
            tc.tile_pool(name="kvsb", bufs=1) as kvsb,
            tc.tile_pool(name="nrp", bufs=2) as nrp,
        ):
            ones_col = cst.tile([128, 1], f16, tag="onesc", name="onesc")
            nc.vector.memset(ones_col[:, :], 1.0)
            eshift = cst.tile([128, 1], f32, tag="esh", name="esh")
            nc.vector.memset(eshift[:, :], ESHIFT)
            epsT = cst.tile([128, 1], f32, tag="eps", name="eps")
            nc.vector.memset(epsT[:, :], EPS)

            # Warmup collective: pays the ncfw cold-start + launch-skew
            # barrier while qkv runs, so the attention-output AllGathers
            # enter the mesh hot. No data deps; transfers garbage.
            nc.gpsimd.collective_compute(
                "AllGather", mybir.AluOpType.bypass,
                ins=[warm_in.ap().opt()], outs=[warm_out.ap().opt()],
                replica_groups=[list(range(NC))],
            )

            rs_cols = cst.tile([128, MT], f32, tag="rscols", name="rscols")
            nc.gpsimd.dma_start(out=rs_cols[:, :], in_=rs_d[:, :])

            # ---- bulk loads, grouped 4 chunks per DMA, interleaved across
            # the three DMA-capable queues so chunk kc=0 lands first.
            # q8/wq live in their own pool that closes after stage C so the
            # o_proj input tiles can reuse that SBUF.
            ldp = tc.tile_pool(name="ldp", bufs=1)
            ldp_pool = ldp.__enter__()
            NG = KCH // 4
            q8g = [ldp_pool.tile([128, 4, S], f16, tag=f"q8g{g}", name=f"q8g{g}")
                   for g in range(NG)]
            wqg = [ldp_pool.tile([128, 4, WQCOLS], f16, tag=f"wqg{g}", name=f"wqg{g}")
                   for g in range(NG)]

            def q8ap(g):
                return q8t_d.ap()[g * 512:(g + 1) * 512, :].rearrange(
                    "(c p) s -> p c s", p=128)

            def wqap(g):
                return wq_d.ap()[g * 512:(g + 1) * 512, :].rearrange(
                    "(c p) s -> p c s", p=128)

            # sync: q8 kc0 first (single chunk), then the rest of group 0
            nc.sync.dma_start(out=q8g[0][:, 0, :], in_=q8ap(0)[:, 0, :])
            nc.scalar.dma_start(out=wqg[0][:, 0, :], in_=wqap(0)[:, 0, :])
            nc.sync.dma_start(out=q8g[0][:, 1:4, :], in_=q8ap(0)[:, 1:4, :])
            nc.scalar.dma_start(out=wqg[0][:, 1:4, :], in_=wqap(0)[:, 1:4, :])
            nc.sync.dma_start(out=q8g[1][:, :, :], in_=q8ap(1))
            nc.gpsimd.dma_start(out=q8g[2][:, :, :], in_=q8ap(2))
            nc.scalar.dma_start(out=wqg[1][:, :, :], in_=wqap(1))
            nc.sync.dma_start(out=q8g[3][:, :, :], in_=q8ap(3))
            nc.gpsimd.dma_start(out=q8g[4][:, :, :], in_=q8ap(4))
            nc.scalar.dma_start(out=wqg[2][:, :, :], in_=wqap(2))
            nc.scalar.dma_start(out=wqg[3][:, :, :], in_=wqap(3))
            nc.scalar.dma_start(out=wqg[4][:, :, :], in_=wqap(4))

            def q8c(kc):
                return q8g[kc // 4][:, kc % 4, :]

            def wqc(kc):
                return wqg[kc // 4][:, kc % 4, :]

            # rope tables / identity / mask on sync, behind the q8 stream;
            # tables are token-major (rope runs pre-transpose in [tok, d])
            tabs = {}
            for nm, d in (("tq1", tq1_d), ("tk1", tk1_d),
                          ("tq2", tq2_d), ("tk2", tk2_d)):
                t = cst.tile([128, MT, HD], f16, tag=f"tb{nm}", name=f"tb{nm}")
                nc.sync.dma_start(out=t[:, :, :],
                                  in_=d.ap().rearrange("(m p) d -> p m d", p=128))
                tabs[nm] = t
            ident = cst.tile([128, 128], f16, tag="id", name="id")
            nc.sync.dma_start(out=ident[:, :], in_=ident_d[:, :])
            cmask = cst.tile([128, 4, 512], f16, tag="cm", name="cm")
            nc.sync.dma_start(out=cmask[:, :, :],
                              in_=cmask_d.ap().rearrange("(r p) n -> p r n", p=128))
            # o_proj weights: not needed until late; tail of scalar queue
            wog = [cst.tile([128, 12, OC], f16, tag=f"wog{g}", name=f"wog{g}")
                   for g in range(2)]
            for g in range(2):
                nc.scalar.dma_start(
                    out=wog[g][:, :, :],
                    in_=wo_d.ap()[g * 1536:(g + 1) * 1536, :].rearrange(
                        "(c p) s -> p c s", p=128))

            def woc(ck):
                return wog[ck // 12][:, ck % 12, :]

            # persistent roped q/k: [d, slot(q0 q1 q2 kA kB), tok]
            qkT = kvsb.tile([128, 5, S], f16, tag="qkT", name="qkT")
            VV = [kvsb.tile([128, 2, 128], f16, tag=f"V{m}", name=f"V{m}")
                  for m in range(MT)]
            # per-k-token exp scale SCALE*rsqrt(ms_k): k rms norm commutes
            # with rope, so it rides the attention exp's per-partition scale
            kscal = [kvsb.tile([128, 2], f32, tag=f"ks{m}", name=f"ks{m}")
                     for m in range(MT)]

            def norm_rope_batched(eng, xn_view, t1, t2, ob_view, scratch_tag):
                """xn_view [128, nh, 128] f16 normalized input in d-permuted
                layout (even dims in cols 0:64, odd in 64:128); t1/t2 f16
                [128, 128] split-table column slices for this m-tile;
                writes roped f16 [128, nh, 128] in the same layout."""
                nh = xn_view.shape[1]
                x0 = xn_view[:, :, 0:64]
                x1 = xn_view[:, :, 64:128]
                t1b = t1.rearrange("p (one d) -> p one d", one=1).to_broadcast(
                    [128, nh, HD])
                t2b = t2.rearrange("p (one d) -> p one d", one=1).to_broadcast(
                    [128, nh, HD])
                a1 = nrp.tile([128, nh, 64], f16, tag=f"ra1{scratch_tag}",
                              name=f"ra1{scratch_tag}")
                a2 = nrp.tile([128, nh, 64], f16, tag=f"ra2{scratch_tag}",
                              name=f"ra2{scratch_tag}")
                eng.tensor_mul(a1[:, :, :], x0, t1b[:, :, 0:64])
                eng.tensor_mul(a2[:, :, :], x1, t2b[:, :, 64:128])
                eng.tensor_sub(ob_view[:, :, 0:64], a1[:, :, :], a2[:, :, :])
                eng.tensor_mul(a1[:, :, :], x0, t2b[:, :, 0:64])
                eng.tensor_mul(a2[:, :, :], x1, t1b[:, :, 64:128])
                eng.tensor_add(ob_view[:, :, 64:128], a1[:, :, :], a2[:, :, :])

            def tcol(nm, m):
                return tabs[nm][:, m, :]

            with (
                tc.tile_pool(name="psq", bufs=3, space="PSUM") as psq,
                tc.tile_pool(name="pst", bufs=2, space="PSUM") as pstp,
            ):
                # ---- Stage C: qkv matmul + norm/rope epilogue + PE
                # transpose into qkT. ACT does ONLY Rsqrt here (evacs are
                # on DVE/gpsimd) so no table thrash.
                def qkv_epilogue(m, psA, psB):
                    rs_ap = rs_cols[:, m:m + 1]
                    psBr = psB.rearrange("p (b c) -> p b c", c=256)
                    qxs = nrp.tile([128, 384], f32, tag="qxs", name="qxs")
                    nc.vector.tensor_copy(qxs[:, :], psA[:, :])
                    kxs = nrp.tile([128, 2, 128], f32, tag="kxs", name="kxs")
                    nc.vector.tensor_copy(kxs[:, :, :], psBr[:, :, 0:128])
                    sq = nrp.tile([128, 384], f32, tag="sqq", name="sqq")
                    sk = nrp.tile([128, 256], f32, tag="sqk", name="sqk")
                    nc.vector.tensor_mul(sq[:, :], qxs[:, :], qxs[:, :])
                    nc.vector.tensor_mul(sk[:, :], kxs.rearrange("p b c -> p (b c)"),
                                         kxs.rearrange("p b c -> p (b c)"))
                    rs5 = nrp.tile([128, 5], f32, tag="rs5", name="rs5")
                    nc.vector.tensor_reduce(rs5[:, 0:3],
                                            sq.rearrange("p (h d) -> p h d", d=128),
                                            mybir.AxisListType.X, mybir.AluOpType.add)
                    nc.vector.tensor_reduce(rs5[:, 3:5],
                                            sk.rearrange("p (h d) -> p h d", d=128),
                                            mybir.AxisListType.X, mybir.AluOpType.add)
                    # rsqrt(ms/HD + eps): DVE fast reciprocal + ACT Sqrt
                    nc.vector.tensor_scalar(rs5[:, :], rs5[:, :], 1.0 / HD, EPS,
                                            mybir.AluOpType.mult,
                                            mybir.AluOpType.add)
                    rc5 = nrp.tile([128, 5], f32, tag="rc5", name="rc5")
                    nc.vector.reciprocal_approx_fast(rc5[:, :], rs5[:, :])
                    nc.scalar.activation(rs5[:, :], rc5[:, :],
                                         mybir.ActivationFunctionType.Sqrt)
                    nc.vector.tensor_scalar_mul(kscal[m][:, :], rs5[:, 3:5], SCALE)
                    # q norm, V scale, and k cast all ride ACT Copy-with-scale
                    # (the DVE is contended during the qkv matmul stream)
                    nc.scalar.activation(VV[m][:, :, :], psBr[:, :, 128:256],
                                         mybir.ActivationFunctionType.Copy,
                                         scale=rs_ap)
                    qx16 = nrp.tile([128, 3, 128], f16, tag="qx16", name="qx16")
                    for h in range(3):
                        nc.scalar.activation(qx16[:, h, :],
                                             qxs[:, h * 128:(h + 1) * 128],
                                             mybir.ActivationFunctionType.Copy,
                                             scale=rs5[:, h:h + 1])
                    kx16 = nrp.tile([128, 2, 128], f16, tag="kx16", name="kx16")
                    nc.scalar.copy(kx16[:, :, :], kxs[:, :, :])
                    rbq = nrp.tile([128, 5, HD], f16, tag="rbq", name="rbq")
                    norm_rope_batched(nc.vector, qx16[:, :, :],
                                      tcol("tq1", m), tcol("tq2", m),
                                      rbq[:, 0:3, :], "q")
                    norm_rope_batched(nc.gpsimd, kx16[:, :, :],
                                      tcol("tk1", m), tcol("tk2", m),
                                      rbq[:, 3:5, :], "k")
                    return rbq

                def transpose_m(m, rbq):
                    # PE transpose [tok, d] -> [d, tok] for the 5 slots,
                    # then one strided DVE evac into qkT columns
                    pst = pstp.tile([128, 5, 128], f16, tag="pst", name="pst")
                    for sl in range(5):
                        nc.tensor.transpose(pst[:, sl, :], rbq[:, sl, :],
                                            ident[:, :])
                    nc.vector.tensor_copy(qkT[:, :, m * 128:(m + 1) * 128],
                                          pst[:, :, :])

                rbqs = {}
                # kc-outer over m0-2 (consumes q8/wq chunks as they arrive)
                psA3 = [psq.tile([128, 384], f32, tag="psA", name="psA")
                        for _ in range(3)]
                psB3 = [psq.tile([128, 512], f32, tag="psB", name="psB")
                        for _ in range(3)]
                for kc in range(KCH):
                    for m in range(3):
                        lh = q8c(kc)[:, m * 128:(m + 1) * 128]
                        nc.tensor.matmul(psA3[m][:, :], lh, wqc(kc)[:, 0:384],
                                         start=(kc == 0), stop=(kc == KCH - 1))
                        nc.tensor.matmul(psB3[m][:, :], lh, wqc(kc)[:, 384:896],
                                         start=(kc == 0), stop=(kc == KCH - 1))
                for m in range(3):
                    rbqs[m] = qkv_epilogue(m, psA3[m], psB3[m])
                # kc-inner for m3-7, transposes of earlier tiles interleaved
                # so the PE never waits on a rope chain
                TSCHED = {3: [0], 4: [1, 2], 5: [3], 6: [4], 7: [5]}
                for m in range(3, MT):
                    psA = psq.tile([128, 384], f32, tag="psA", name="psA")
                    psB = psq.tile([128, 512], f32, tag="psB", name="psB")
                    for kc in range(KCH):
                        lh = q8c(kc)[:, m * 128:(m + 1) * 128]
                        nc.tensor.matmul(psA[:, :], lh, wqc(kc)[:, 0:384],
                                         start=(kc == 0), stop=(kc == KCH - 1))
                        nc.tensor.matmul(psB[:, :], lh, wqc(kc)[:, 384:896],
                                         start=(kc == 0), stop=(kc == KCH - 1))
                    for tm in TSCHED[m]:
                        transpose_m(tm, rbqs.pop(tm))
                    rbqs[m] = qkv_epilogue(m, psA, psB)
                for m in sorted(rbqs):
                    transpose_m(m, rbqs.pop(m))
            ldp.__exit__(None, None, None)

            # ---- Stage F: attention, 512-wide q groups, scoresT [k, q];
            # grp0 (tokens 0-511) first so its AllGather fires early and
            # o_proj-A overlaps AG_b's mesh. ACT does ONLY Exp here; 1/den
            # via DVE reciprocal_approx_fast; broadcast via gpsimd.
            with (
                tc.tile_pool(name="pssc", bufs=4, space="PSUM") as pssc,
                tc.tile_pool(name="psav", bufs=2, space="PSUM") as psav,
                tc.tile_pool(name="psden", bufs=1, space="PSUM") as psden,
                tc.tile_pool(name="pso", bufs=1, space="PSUM") as pso,
                tc.tile_pool(name="ptt", bufs=12) as ptt,
                tc.tile_pool(name="accp", bufs=2) as accp,
                tc.tile_pool(name="qga", bufs=2) as qga,
                tc.tile_pool(name="agtp", bufs=2 * NH) as agtp,
                tc.tile_pool(name="ogp", bufs=2) as ogp,
            ):
                agt_a = []
                agt_b = []
                for grp in (0, 1):
                    gs = slice(grp * 512, grp * 512 + 512)
                    nkc = 4 * grp + 4
                    for sl in range(3):
                        blk = 0 if sl < 2 else 1
                        pts = []
                        den_ps = psden.tile([1, 512], f32, tag="den", name="den")
                        # all score matmuls first — exps chase them via psum
                        # recycling, so the PE never waits on the exp chain
                        for kc in range(nkc):
                            ps = pssc.tile([128, 512], f32, tag="sc", name="sc")
                            nc.tensor.matmul(ps[:, :],
                                             qkT[:, 3 + blk, kc * 128:(kc + 1) * 128],
                                             qkT[:, sl, gs], start=True, stop=True)
                            pt = ptt.tile([128, 512], f16, tag="pt", name="pt")
                            nc.scalar.activation(pt[:, :], ps[:, :],
                                                 mybir.ActivationFunctionType.Exp,
                                                 bias=eshift[:, 0:1],
                                                 scale=kscal[kc][:, blk:blk + 1])
                            r = kc - 4 * grp
                            if r >= 0:
                                nc.vector.tensor_mul(pt[:, :], pt[:, :], cmask[:, r, :])
                            pts.append(pt)
                        avp = psav.tile([128, 512], f32, tag="av", name="av")
                        for kc in range(nkc):
                            nc.tensor.matmul(den_ps[:, :], ones_col[:, :],
                                             pts[kc][:, :],
                                             start=(kc == 0), stop=(kc == nkc - 1))
                            nc.tensor.matmul(avp[:, :], VV[kc][:, blk, :],
                                             pts[kc][:, :],
                                             start=(kc == 0), stop=(kc == nkc - 1))
                        # clamp away 0/denorm (undefined for the fast recip)
                        dsb = accp.tile([1, 512], f32, tag="dsb", name="dsb")
                        nc.vector.tensor_scalar_max(dsb[:, :], den_ps[:, :], 1e-20)
                        rden = accp.tile([1, 512], f32, tag="rdr", name="rdr")
                        nc.vector.reciprocal_approx_fast(rden[:, :], dsb[:, :])
                        fac = qga.tile([128, 512], f32, tag="fac", name="fac")
                        nc.gpsimd.partition_broadcast(fac[:, :], rden[:, :])
                        aq = qga.tile([128, 512], f16, tag="aq", name="aq")
                        nc.vector.tensor_mul(aq[:, :], avp[:, :], fac[:, :])
                        # aq outputs ride the gpsimd queue (same as the AG
                        # triggers) so agt loads on sync/scalar can't delay
                        # this core's collective inputs
                        if grp == 1:
                            nc.gpsimd.dma_start(
                                out=agin_b[sl * 128:(sl + 1) * 128, :],
                                in_=aq[:, :])
                        elif sl < 2:
                            nc.gpsimd.dma_start(
                                out=agin_a1[sl * 128:(sl + 1) * 128, :],
                                in_=aq[:, :])
                        else:
                            nc.gpsimd.dma_start(out=agin_a2[:, :], in_=aq[:, :])
                        if grp == 0 and sl == 1:
                            nc.gpsimd.collective_compute(
                                "AllGather", mybir.AluOpType.bypass,
                                ins=[agin_a1.ap().opt()],
                                outs=[agout_a1.ap().opt()],
                                replica_groups=[list(range(NC))],
                            )
                    nc.gpsimd.collective_compute(
                        "AllGather", mybir.AluOpType.bypass,
                        ins=[(agin_b if grp == 1 else agin_a2).ap().opt()],
                        outs=[(agout_b if grp == 1 else agout_a2).ap().opt()],
                        replica_groups=[list(range(NC))],
                    )
                    # issue half-A o_proj input loads right behind its AG
                    # trigger so they land the moment the mesh finishes.
                    # Half-B loads are deferred past the half-A matmuls:
                    # queue-position DMA semaphores would otherwise make
                    # o_proj-A wait for AG_b's mesh.
                    if grp == 0:
                        CH01 = [j * 3 + s for j in range(NC) for s in range(2)]
                        CH2 = [j * 3 + 2 for j in range(NC)
                               if HEADS[j][2] is not None]
                        for ci, ck in enumerate(CH01 + CH2):
                            t = agtp.tile([128, 512], f16, tag="agt", name="agt")
                            deng = nc.sync if ci % 2 == 0 else nc.scalar
                            cj, cs = divmod(ck, 3)
                            if cs < 2:
                                ro = cj * 256 + cs * 128
                                src = agout_a1[ro:ro + 128, :]
                            else:
                                src = agout_a2[cj * 128:(cj + 1) * 128, :]
                            deng.dma_start(out=t[:, :], in_=src)
                            agt_a.append((ck, t))

                # ---- o_proj: half A (tokens 0-511) first — its inputs
                # arrived during grp1 attention; half B rides AG_b.
                def oproj_half(hf, agt):
                    for j in range(4):
                        m = hf * 4 + j
                        ps = pso.tile([128, OC], f32, tag="po", name="po")
                        for i, (ck, t) in enumerate(agt):
                            nc.tensor.matmul(ps[:, :],
                                             t[:, j * 128:(j + 1) * 128],
                                             woc(ck)[:, :],
                                             start=(i == 0),
                                             stop=(i == NH - 1))
                        og = ogp.tile([128, OC], f32, tag="og", name="og")
                        nc.scalar.copy(og[:, :], ps[:, :])
                        nc.sync.dma_start(out=out_d[m * 128:(m + 1) * 128, :],
                                          in_=og[:, :])

                oproj_half(0, agt_a)
                for ci, ck in enumerate(REAL_CHUNKS):
                    t = agtp.tile([128, 512], f16, tag="agt", name="agt")
                    deng = nc.sync if ci % 2 == 0 else nc.scalar
                    deng.dma_start(out=t[:, :],
                                   in_=agout_b[ck * 128:(ck + 1) * 128, :])
                    agt_b.append((ck, t))
                oproj_half(1, agt_b)

    nc.compile()
    return nc


def _host_prep(x, w_qkv, ws_qkv, w_o, ws_o, q_norm_w, k_norm_w):
    w_dq = (w_qkv * np.repeat(ws_qkv, GS, axis=1)).astype(np.float32)
    wo_dq = (w_o * np.repeat(ws_o, GS, axis=1)).astype(np.float32)

    # per-token int8 activation quant (reference's _quant_act), transposed
    # to [dim, tok] so the device consumes it without DMA transposes
    mx = np.clip(np.max(np.abs(x), axis=-1, keepdims=True), 1e-5, None)
    q8 = np.clip(np.round(x * (127.0 / mx)), -128.0, 127.0)
    q8t = np.ascontiguousarray(q8.T).astype(FP16)            # [DIM, S]
    rs = np.ascontiguousarray(
        (mx[:, 0] / 127.0).reshape(MT, 128).T).astype(np.float32)  # [128, MT]

    pos = np.arange(S, dtype=np.float32)
    inv_freq = (THETA ** (-np.arange(0, HD, 2, dtype=np.float32) / HD)).astype(np.float32)
    ang = pos[:, None] * inv_freq[None, :]
    co = np.cos(ang).astype(np.float32)                  # [S, 64]
    si = np.sin(ang).astype(np.float32)
    # split rope tables for the d-permuted (even|odd) head layout:
    # t1 = [c*w_even | c*w_odd], t2 = [s*w_even | s*w_odd]
    tq1 = np.concatenate([co * q_norm_w[0::2], co * q_norm_w[1::2]], 1).astype(FP16)
    tq2 = np.concatenate([si * q_norm_w[0::2], si * q_norm_w[1::2]], 1).astype(FP16)
    tk1 = np.concatenate([co * k_norm_w[0::2], co * k_norm_w[1::2]], 1).astype(FP16)
    tk2 = np.concatenate([si * k_norm_w[0::2], si * k_norm_w[1::2]], 1).astype(FP16)
    # even dims then odd dims within each q/k head (scores are invariant
    # since q and k share the permutation; v / o_proj stay unpermuted)
    dperm = np.concatenate([np.arange(0, HD, 2), np.arange(1, HD, 2)])

    ident = np.eye(128, dtype=FP16)

    # mask variants: scoresT [k(128), 512 q]; group cols = 4 q-blocks; r = kc-4*grp
    cm = np.zeros((4, 128, 512), np.float32)
    tri = np.triu(np.ones((128, 128), np.float32))  # keep k <= q
    for r in range(4):
        for j in range(4):
            if j > r:
                cm[r, :, j * 128:(j + 1) * 128] = 1.0
            elif j == r:
                cm[r, :, j * 128:(j + 1) * 128] = tri
    cmask = cm.reshape(4 * 128, 512).astype(FP16)

    in_maps = []
    for c in range(NC):
        wq = np.zeros((DIM, WQCOLS), np.float32)
        for sl in range(3):
            h = HEADS[c][sl]
            if h is not None:
                wq[:, sl * 128:(sl + 1) * 128] = w_dq[h * HD + dperm, :].T
        ga = GA[c]
        wq[:, 384:512] = w_dq[KBASE + ga * HD + dperm, :].T
        wq[:, 512:640] = w_dq[VBASE + ga * HD:VBASE + (ga + 1) * HD, :].T
        gb = GB[c]
        if gb is not None:
            wq[:, 640:768] = w_dq[KBASE + gb * HD + dperm, :].T
            wq[:, 768:896] = w_dq[VBASE + gb * HD:VBASE + (gb + 1) * HD, :].T

        wo = np.zeros((NC * 384, OC), np.float32)
        for j in range(NC):
            for sl in range(3):
                h = HEADS[j][sl]
                if h is not None:
                    rws = slice((j * 3 + sl) * 128, (j * 3 + sl) * 128 + 128)
                    wo[rws, :] = wo_dq[c * OC:(c + 1) * OC, h * HD:(h + 1) * HD].T

        in_maps.append({
            "q8t": q8t,
            "rs": rs,
            "wq": wq.astype(FP16),
            "wo": wo.astype(FP16),
            "tq1": tq1, "tq2": tq2, "tk1": tk1, "tk2": tk2,
            "ident": ident,
            "cmask": cmask,
        })
    return in_maps


def kernel(x, w_qkv, ws_qkv, w_o, ws_o, q_norm_w, k_norm_w):
    x = np.asarray(x, np.float32)
    w_qkv = np.asarray(w_qkv, np.float32)
    ws_qkv = np.asarray(ws_qkv, np.float32)
    w_o = np.asarray(w_o, np.float32)
    ws_o = np.asarray(ws_o, np.float32)
    q_norm_w = np.asarray(q_norm_w, np.float32)
    k_norm_w = np.asarray(k_norm_w, np.float32)

    if "nc" not in _cached:
        _cached["nc"] = _build_nc()
    nc = _cached["nc"]

    in_maps = _host_prep(x, w_qkv, ws_qkv, w_o, ws_o, q_norm_w, k_norm_w)
    trace = bool(int(os.environ.get("BENCH_TRACE", "0")))
    res = run_bass_kernel_spmd(nc, in_maps, core_ids=list(range(NC)), trace=trace)
    _cached["res"] = res
    if trace and res.exec_time_ns is not None:
        print(f"HW exec time: {res.exec_time_ns} ns")
        _cached["exec_time_ns"] = res.exec_time_ns

    out = np.concatenate([np.asarray(res.results[c]["out"], np.float32)
                          for c in range(NC)], axis=1)
    return out
